# revision 30
# baseline (speedup 1.0000x reference)
"""Contrastive pairwise-margin loss on 8 Trainium2 NeuronCores.

loss = sum_{i,j} [ R_ij * d_ij + (1-R_ij) * relu(0.5 - d_ij) ] / (N*(N-1)*2)
with d_ij = ||x_i - x_j||^2 and R_ij = [t_i == t_j].

Decomposition (host rows sorted by class):
  loss_sum = sum_{i!=j} relu(m - d_ij)  +  sum_{same-class, i!=j} [d - relu(m-d)]
The second term is exact fp64 on the host (O(sum n_c^2 * D), tiny). The device
computes only the uniform all-pairs relu sum over the upper-triangle blocks:
  relu(m - d_ij) = 2 * relu(a_ij),  a_ij = g_ij + h_j + c_i
  g = x_i.x_j (fp8 gram), h_j = -sq_j/2, c_i = (m - sq_i)/2.
Per [128,<=512] tile: 1 fp8 DoubleRow gram matmul + 1 fp8 DoubleRow "aug"
matmul (Ki=2: rows 1*h_hi, 1*h_lo, c_hi*1, c_lo*1) accumulate a into PSUM;
relu+row-sum is one fused op per 2-bank PSUM group, split between VectorE
(tensor_scalar max-0 / add-reduce) and ScalarE (activation Relu + accum_out)
by a static cost balance. Diagonal blocks compute only their upper triangle
(per-mi column offsets) and reuse the lhs operand as rhs. Host: everything
x4 (2 from relu(2a)=2relu(a), 2 from block symmetry), minus 4*relu(a_ii)
(device diagonal entries, emulated exactly), plus the exact same-class term.
"""

import os
import sys

for _p in ("/opt/trn_rl_repo", "/root/.axon_site/_ro/trn_rl_repo"):
    if os.path.isdir(_p) and _p not in sys.path:
        sys.path.insert(0, _p)

from contextlib import ExitStack

import ml_dtypes
import numpy as np

import concourse.bass as bass  # noqa: F401
import concourse.mybir as mybir
from concourse import bacc, bass_utils
from concourse.tile import TileContext

FP8 = ml_dtypes.float8_e4m3
MARGIN = 0.5
N = 8192
D = 256
P = 128
BLK = 512
NBLK = N // BLK        # 16 stripes
NCORES = 8
NBLOCKS = 17           # blocks per core (2 diag + 15 off-diag)
NGROUPS = NBLOCKS * 2  # relu ops per core
NWARM = 6              # PE p-state warm-up matmuls

# operand slots: 0,1 = diag blocks (lhs==rhs); off-diag block i (pos 2..16)
# has lhs slot 2+2*(i-2), rhs slot 3+2*(i-2)  -> 2 + 15*2 = 32 slots
NSLOTS = 2 + 15 * 2

# DMA chunking in slot units (first chunks small so compute starts early)
SLOT_CHUNKS = [1, 1, 2, 4, 6, 8, 6, 4]
assert sum(SLOT_CHUNKS) == NSLOTS

# group table: (block_pos, [(mi, rhs_off, width, psum_off)...], width, wgt)
# diag blocks: strict-upper tile slices (weight 4) + the four diagonal
# 128x128 sub-squares packed into one 512-wide group (weight 2).
GROUPS = []
for _pos in range(2):
    GROUPS.append((_pos, [(0, 128, 384, 0)], 384, 4.0))
    GROUPS.append((_pos, [(1, 256, 256, 0), (2, 384, 128, 256)], 384, 4.0))
    GROUPS.append((_pos, [(mi, mi * P, P, mi * P) for mi in range(4)],
                   512, 2.0))
for _pos in range(2, NBLOCKS):
    GROUPS.append((_pos, [(0, 0, BLK, 0), (1, 0, BLK, BLK)], 1024, 4.0))
    GROUPS.append((_pos, [(2, 0, BLK, 0), (3, 0, BLK, BLK)], 1024, 4.0))

# static engine split balanced by modeled op cost (+ per-op seq/sem bias)
ACT_BIAS = float(os.environ.get("K_ACT_BIAS", "0"))

def _op_cost(width, eng):
    if eng == "dve":
        return (width + 120) * 1.0417
    return (width + 222) * 0.8333 + 187 + ACT_BIAS

def _assign_engines():
    dve_t = act_t = 0.0
    out = []
    for _, _, w, _wgt in GROUPS:
        cd, ca = _op_cost(w, "dve"), _op_cost(w, "act")
        if dve_t + cd <= act_t + ca:
            out.append("dve")
            dve_t += cd
        else:
            out.append("act")
            act_t += ca
    return out

ENGINES = _assign_engines()
DVE_COLS = [i for i, e in enumerate(ENGINES) if e == "dve"]
ACC_COL = {}
for _i, _g in enumerate(DVE_COLS):
    ACC_COL[_g] = ("dve", _i)
_ACT_COLS = [i for i, e in enumerate(ENGINES) if e == "act"]
for _i, _g in enumerate(_ACT_COLS):
    ACC_COL[_g] = ("act", _i)
N_DVE = len(DVE_COLS)
N_ACT = len(_ACT_COLS)
W_DVE = np.array([GROUPS[g][3] for g in DVE_COLS])
W_ACT = np.array([GROUPS[g][3] for g in _ACT_COLS])

_CHUNK_OF_SLOT = {}
_c0 = 0
for _g, _n in enumerate(SLOT_CHUNKS):
    for _s in range(_c0, _c0 + _n):
        _CHUNK_OF_SLOT[_s] = (_g, _s - _c0)
    _c0 += _n

def _pos_slots(pos):
    """(lhs_slot, rhs_slot) for block position."""
    if pos < 2:
        return pos, pos
    return 2 + 2 * (pos - 2), 3 + 2 * (pos - 2)

_COMPILED = None
LAST_RESULTS = None


def _build_program():
    nc = bacc.Bacc("TRN2", target_bir_lowering=False, debug=False,
                   num_devices=NCORES)
    f8 = mybir.dt.float8e4
    bf = mybir.dt.bfloat16
    f32 = mybir.dt.float32
    Alu = mybir.AluOpType
    Relu = mybir.ActivationFunctionType.Relu

    ops_d = nc.dram_tensor("ops", [P, NSLOTS * 2, BLK], f8,
                           kind="ExternalInput")
    aug_d = nc.dram_tensor("aug", [2, NBLOCKS * 4, BLK], f8,
                           kind="ExternalInput")
    accd_d = nc.dram_tensor("accd", [P, N_DVE], f32, kind="ExternalOutput")
    acca_d = nc.dram_tensor("acca", [P, N_ACT], f32, kind="ExternalOutput")

    with TileContext(nc) as tc, ExitStack() as ctx:
        sb = ctx.enter_context(tc.tile_pool(name="sb", bufs=1))
        wpool = ctx.enter_context(tc.tile_pool(name="wpool", bufs=2))
        pp = ctx.enter_context(tc.tile_pool(name="pp", bufs=4, space="PSUM"))

        aug_t = sb.tile([2, NBLOCKS * 4, BLK], f8)
        accd_t = sb.tile([P, N_DVE], f32)
        acca_t = sb.tile([P, N_ACT], f32)

        # first chunks go through SWDGE (gpsimd queue) in parallel with the
        # HWDGE pipe that carries aug + the rest; both start immediately
        slots_g = []
        c0 = 0
        chunk_tiles = []
        for g, nb in enumerate(SLOT_CHUNKS):
            bt = sb.tile([P, nb * 2, BLK], f8, tag=f"ops{g}")
            chunk_tiles.append((g, nb, bt))
            slots_g.append((c0, bt))
            c0 += nb
        c0 = 0
        for g, nb, bt in chunk_tiles:
            q = nc.gpsimd if g < 2 else nc.sync
            if g == 2:
                nc.sync.dma_start(aug_t[:], aug_d[:])
            q.dma_start(bt[:], ops_d[:, c0 * 2:(c0 + nb) * 2, :])
            c0 += nb

        # warm the ACT table set while DMAs ramp (hides LoadActFuncSet)
        warm = wpool.tile([P, 1], f32, tag="warm")
        nc.vector.memset(warm[:], 0.0)
        nc.scalar.activation(warm[:], warm[:], Relu)

        # PE p-state warm-up: chain of cheap matmuls on a zeroed operand
        wop = wpool.tile([P, 2, 256], f8, tag="wop")
        nc.vector.memset(wop[:], 0.0)
        wp = pp.tile([P, 2 * BLK], f32, tag="p")
        DR = mybir.MatmulPerfMode.DoubleRow
        for _ in range(NWARM):
            nc.tensor.matmul(wp[:, 0:256], wop[:, :, 0:P], wop[:],
                             start=True, stop=True, perf_mode=DR)

        def slot_ap(s):
            g, off = _CHUNK_OF_SLOT[s]
            bt = slots_g[g][1]
            return bt[:, off * 2:off * 2 + 2, :]   # [128, 2, 512]

        for grp, (pos, parts, width, _wgt) in enumerate(GROUPS):
            ls, rs = _pos_slots(pos)
            lhs3 = slot_ap(ls)
            rhs3 = slot_ap(rs)
            augL = aug_t[:, pos * 4:pos * 4 + 2, :]
            augR = aug_t[:, pos * 4 + 2:pos * 4 + 4, :]
            p_t = pp.tile([P, 2 * BLK], f32, tag="p")
            for mi, off, w, pcol in parts:
                sl = slice(pcol, pcol + w)
                nc.tensor.matmul(p_t[:, sl],
                                 lhs3[:, :, mi * P:(mi + 1) * P],
                                 rhs3[:, :, off:off + w],
                                 start=True, stop=False, perf_mode=DR)
                nc.tensor.matmul(p_t[:, sl],
                                 augL[:, :, mi * P:(mi + 1) * P],
                                 augR[:, :, off:off + w],
                                 start=False, stop=True, perf_mode=DR)
            eng, acol = ACC_COL[grp]
            if eng == "dve":
                nc.vector.tensor_scalar(p_t[:, :width], p_t[:, :width],
                                        0.0, 0.0, op0=Alu.max, op1=Alu.add,
                                        accum_out=accd_t[:, acol:acol + 1])
            else:
                nc.scalar.activation(p_t[:, :width], p_t[:, :width], Relu,
                                     accum_out=acca_t[:, acol:acol + 1])

        # split outputs: bulk prefix overlaps compute; tiny suffix after the
        # engine's last op
        nc.sync.dma_start(accd_d[:], accd_t[:])
        nc.gpsimd.dma_start(acca_d[:], acca_t[:])

    nc.compile()
    return nc


def _get_program():
    global _COMPILED
    if _COMPILED is None:
        _COMPILED = _build_program()
    return _COMPILED


def _core_blocks(k):
    """17 (row, col) upper-tri blocks for core k; the 2 diagonal first."""
    ra, rb = k, NBLK - 1 - k
    order = [(ra, ra), (rb, rb)]
    order += [(ra, c) for c in range(ra + 1, NBLK)]
    order += [(rb, c) for c in range(rb + 1, NBLK)]
    assert len(order) == NBLOCKS
    return order


def kernel(inputs: np.ndarray, target: np.ndarray) -> np.ndarray:
    global LAST_RESULTS
    x = np.asarray(inputs, dtype=np.float32)
    t = np.asarray(target).astype(np.int64)
    assert x.shape == (N, D) and t.shape == (N,)

    perm = np.argsort(t, kind="stable")
    xs = x[perm]
    ts = t[perm]

    x8 = xs.astype(FP8)                       # [N, 256]
    x8f = x8.astype(np.float32)
    sq = (xs.astype(np.float64) ** 2).sum(axis=1).astype(np.float32)

    h = (-0.5 * sq).astype(np.float32)                      # -sq_j / 2
    c = (0.5 * (MARGIN - sq)).astype(np.float32)            # (m - sq_i)/2
    h_hi = h.astype(FP8)
    h_lo = (h - h_hi.astype(np.float32)).astype(FP8)
    c_hi = c.astype(FP8)
    c_lo = (c - c_hi.astype(np.float32)).astype(FP8)

    # packed operand [128, 2, N]: xop[p, s, n] = x8[n, s*128 + p]
    xop = np.ascontiguousarray(x8.T.reshape(2, P, N).transpose(1, 0, 2))

    in_maps = []
    for k in range(NCORES):
        order = _core_blocks(k)
        ops = np.empty((P, NSLOTS * 2, BLK), FP8)
        aug = np.zeros((2, NBLOCKS * 4, BLK), FP8)
        for pos, (r, cb) in enumerate(order):
            rsl = slice(r * BLK, (r + 1) * BLK)
            csl = slice(cb * BLK, (cb + 1) * BLK)
            ls, rs = _pos_slots(pos)
            ops[:, 2 * ls:2 * ls + 2, :] = xop[:, :, rsl]
            if rs != ls:
                ops[:, 2 * rs:2 * rs + 2, :] = xop[:, :, csl]
            aug[:, 4 * pos, :] = 1.0
            aug[0, 4 * pos + 1, :] = c_hi[rsl]
            aug[1, 4 * pos + 1, :] = c_lo[rsl]
            aug[0, 4 * pos + 2, :] = h_hi[csl]
            aug[1, 4 * pos + 2, :] = h_lo[csl]
            aug[:, 4 * pos + 3, :] = 1.0
        in_maps.append({"ops": ops, "aug": aug})

    nc = _get_program()
    res = bass_utils.run_bass_kernel_spmd(
        nc, in_maps, core_ids=list(range(NCORES)))
    LAST_RESULTS = res

    total = 0.0
    for k in range(NCORES):
        accd = res.results[k]["accd"].astype(np.float64)
        acca = res.results[k]["acca"].astype(np.float64)
        total += (accd.sum(axis=0) * W_DVE).sum()
        total += (acca.sum(axis=0) * W_ACT).sum()

    # exact diagonal-entry removal (device computed i==j in the weight-2
    # diagonal sub-square groups)
    g_ii = (x8f * x8f).sum(axis=1, dtype=np.float32)
    a_ii = (g_ii
            + (h_hi.astype(np.float32) + h_lo.astype(np.float32))
            + (c_hi.astype(np.float32) + c_lo.astype(np.float32)))
    total -= 2.0 * np.maximum(a_ii, 0.0).astype(np.float64).sum()

    # exact same-class term in fp64: sum_{same, i != j} [d - relu(m - d)]
    sq64 = (xs.astype(np.float64) ** 2).sum(axis=1)
    nclasses = int(ts.max()) + 1
    counts = np.bincount(ts, minlength=nclasses)
    starts = np.concatenate([[0], np.cumsum(counts)])
    for cc in range(nclasses):
        lo, hi = starts[cc], starts[cc + 1]
        if hi - lo < 2:
            continue
        Xc = xs[lo:hi].astype(np.float64)
        sqc = sq64[lo:hi]
        dm = sqc[:, None] + sqc[None, :] - 2.0 * (Xc @ Xc.T)
        np.fill_diagonal(dm, np.nan)
        total += np.nansum(dm) - np.nansum(np.maximum(MARGIN - dm, 0.0))

    loss = total / (N * (N - 1.0) * 2.0)
    return np.float32(loss)


# revision 31
# speedup vs baseline: 1.0744x; 1.0744x over previous
"""Contrastive pairwise-margin loss on 8 Trainium2 NeuronCores.

loss = sum_{i,j} [ R_ij * d_ij + (1-R_ij) * relu(0.5 - d_ij) ] / (N*(N-1)*2)
with d_ij = ||x_i - x_j||^2 and R_ij = [t_i == t_j].

Decomposition (host rows sorted by class):
  loss_sum = sum_{i!=j} relu(m - d_ij)  +  sum_{same-class, i!=j} [d - relu(m-d)]
The second term is exact fp64 on the host (O(sum n_c^2 * D), tiny). The device
computes only the uniform all-pairs relu sum over the upper-triangle blocks:
  relu(m - d_ij) = 2 * relu(a_ij),  a_ij = g_ij + h_j + c_i
  g = x_i.x_j (fp8 gram), h_j = -sq_j/2, c_i = (m - sq_i)/2.
Per [128,<=512] tile: 1 fp8 DoubleRow gram matmul + 1 fp8 DoubleRow "aug"
matmul (Ki=2: rows 1*h_hi, 1*h_lo, c_hi*1, c_lo*1) accumulate a into PSUM;
relu+row-sum is one fused op per 2-bank PSUM group, split between VectorE
(tensor_scalar max-0 / add-reduce) and ScalarE (activation Relu + accum_out)
by a static cost balance. Diagonal blocks compute only their upper triangle
(per-mi column offsets) and reuse the lhs operand as rhs. Host: everything
x4 (2 from relu(2a)=2relu(a), 2 from block symmetry), minus 4*relu(a_ii)
(device diagonal entries, emulated exactly), plus the exact same-class term.
"""

import os
import sys

for _p in ("/opt/trn_rl_repo", "/root/.axon_site/_ro/trn_rl_repo"):
    if os.path.isdir(_p) and _p not in sys.path:
        sys.path.insert(0, _p)

from contextlib import ExitStack

import ml_dtypes
import numpy as np

import concourse.bass as bass  # noqa: F401
import concourse.mybir as mybir
from concourse import bacc, bass_utils
from concourse.tile import TileContext

FP8 = ml_dtypes.float8_e4m3
MARGIN = 0.5
N = 8192
D = 256
P = 128
BLK = 512
NBLK = N // BLK        # 16 stripes
NCORES = 8
NBLOCKS = 17           # blocks per core (2 diag + 15 off-diag)
NGROUPS = NBLOCKS * 2  # relu ops per core
NWARM = 6              # PE p-state warm-up matmuls

# operand slots: 0,1 = diag blocks (lhs==rhs); off-diag block i (pos 2..16)
# has lhs slot 2+2*(i-2), rhs slot 3+2*(i-2)  -> 2 + 15*2 = 32 slots
NSLOTS = 2 + 15 * 2

# DMA chunking in slot units (first chunks small so compute starts early)
SLOT_CHUNKS = [1, 1, 2, 4, 6, 8, 6, 4]
assert sum(SLOT_CHUNKS) == NSLOTS

# group table: (block_pos, [(mi, rhs_off, width, psum_off)...], width, wgt)
# diag blocks: strict-upper tile slices (weight 4) + the four diagonal
# 128x128 sub-squares packed into one 512-wide group (weight 2).
GROUPS = []
for _pos in range(2):
    GROUPS.append((_pos, [(0, 128, 384, 0)], 384, 4.0))
    GROUPS.append((_pos, [(1, 256, 256, 0), (2, 384, 128, 256)], 384, 4.0))
    GROUPS.append((_pos, [(mi, mi * P, P, mi * P) for mi in range(4)],
                   512, 2.0))
for _pos in range(2, NBLOCKS):
    GROUPS.append((_pos, [(0, 0, BLK, 0), (1, 0, BLK, BLK)], 1024, 4.0))
    GROUPS.append((_pos, [(2, 0, BLK, 0), (3, 0, BLK, BLK)], 1024, 4.0))

# static engine split balanced by modeled op cost (+ per-op seq/sem bias)
ACT_BIAS = float(os.environ.get("K_ACT_BIAS", "0"))

def _op_cost(width, eng):
    if eng == "dve":
        return (width + 120) * 1.0417
    return (width + 222) * 0.8333 + 187 + ACT_BIAS

def _assign_engines():
    dve_t = act_t = 0.0
    out = []
    for _, _, w, _wgt in GROUPS:
        cd, ca = _op_cost(w, "dve"), _op_cost(w, "act")
        if dve_t + cd <= act_t + ca:
            out.append("dve")
            dve_t += cd
        else:
            out.append("act")
            act_t += ca
    return out

ENGINES = _assign_engines()
DVE_COLS = [i for i, e in enumerate(ENGINES) if e == "dve"]
ACC_COL = {}
for _i, _g in enumerate(DVE_COLS):
    ACC_COL[_g] = ("dve", _i)
_ACT_COLS = [i for i, e in enumerate(ENGINES) if e == "act"]
for _i, _g in enumerate(_ACT_COLS):
    ACC_COL[_g] = ("act", _i)
N_DVE = len(DVE_COLS)
N_ACT = len(_ACT_COLS)
W_DVE = np.array([GROUPS[g][3] for g in DVE_COLS])
W_ACT = np.array([GROUPS[g][3] for g in _ACT_COLS])

_CHUNK_OF_SLOT = {}
_c0 = 0
for _g, _n in enumerate(SLOT_CHUNKS):
    for _s in range(_c0, _c0 + _n):
        _CHUNK_OF_SLOT[_s] = (_g, _s - _c0)
    _c0 += _n

def _pos_slots(pos):
    """(lhs_slot, rhs_slot) for block position."""
    if pos < 2:
        return pos, pos
    return 2 + 2 * (pos - 2), 3 + 2 * (pos - 2)

_COMPILED = None
LAST_RESULTS = None


def _build_program():
    nc = bacc.Bacc("TRN2", target_bir_lowering=False, debug=False,
                   num_devices=NCORES)
    f8 = mybir.dt.float8e4
    bf = mybir.dt.bfloat16
    f32 = mybir.dt.float32
    Alu = mybir.AluOpType
    Relu = mybir.ActivationFunctionType.Relu

    ops_d = nc.dram_tensor("ops", [P, NSLOTS * 2, BLK], f8,
                           kind="ExternalInput")
    aug_d = nc.dram_tensor("aug", [2, NBLOCKS * 4, BLK], f8,
                           kind="ExternalInput")
    accd_d = nc.dram_tensor("accd", [P, N_DVE], f32, kind="ExternalOutput")
    acca_d = nc.dram_tensor("acca", [P, N_ACT], f32, kind="ExternalOutput")

    with TileContext(nc) as tc, ExitStack() as ctx:
        sb = ctx.enter_context(tc.tile_pool(name="sb", bufs=1))
        wpool = ctx.enter_context(tc.tile_pool(name="wpool", bufs=2))
        pp = ctx.enter_context(tc.tile_pool(name="pp", bufs=4, space="PSUM"))

        aug_t = sb.tile([2, NBLOCKS * 4, BLK], f8)
        accd_t = sb.tile([P, N_DVE], f32)
        acca_t = sb.tile([P, N_ACT], f32)

        # warm the ACT table set while DMAs ramp (hides LoadActFuncSet)
        warm = wpool.tile([P, 1], f32, tag="warm")
        nc.gpsimd.memset(warm[:], 0.0)
        nc.scalar.activation(warm[:], warm[:], Relu)

        # PE p-state warm-up: chain of cheap matmuls on a zeroed operand
        wop = wpool.tile([P, 2, 256], f8, tag="wop")
        nc.gpsimd.memset(wop[:], 0.0)
        wp = pp.tile([P, 2 * BLK], f32, tag="p")
        DR = mybir.MatmulPerfMode.DoubleRow
        for _ in range(NWARM):
            nc.tensor.matmul(wp[:, 0:256], wop[:, :, 0:P], wop[:],
                             start=True, stop=True, perf_mode=DR)

        nc.sync.dma_start(aug_t[:], aug_d[:])

        slots_g = []
        c0 = 0
        for g, nb in enumerate(SLOT_CHUNKS):
            bt = sb.tile([P, nb * 2, BLK], f8, tag=f"ops{g}")
            nc.sync.dma_start(bt[:], ops_d[:, c0 * 2:(c0 + nb) * 2, :])
            slots_g.append((c0, bt))
            c0 += nb

        def slot_ap(s):
            g, off = _CHUNK_OF_SLOT[s]
            bt = slots_g[g][1]
            return bt[:, off * 2:off * 2 + 2, :]   # [128, 2, 512]

        for grp, (pos, parts, width, _wgt) in enumerate(GROUPS):
            ls, rs = _pos_slots(pos)
            lhs3 = slot_ap(ls)
            rhs3 = slot_ap(rs)
            augL = aug_t[:, pos * 4:pos * 4 + 2, :]
            augR = aug_t[:, pos * 4 + 2:pos * 4 + 4, :]
            p_t = pp.tile([P, 2 * BLK], f32, tag="p")
            for mi, off, w, pcol in parts:
                sl = slice(pcol, pcol + w)
                nc.tensor.matmul(p_t[:, sl],
                                 lhs3[:, :, mi * P:(mi + 1) * P],
                                 rhs3[:, :, off:off + w],
                                 start=True, stop=False, perf_mode=DR)
                nc.tensor.matmul(p_t[:, sl],
                                 augL[:, :, mi * P:(mi + 1) * P],
                                 augR[:, :, off:off + w],
                                 start=False, stop=True, perf_mode=DR)
            eng, acol = ACC_COL[grp]
            if eng == "dve":
                nc.vector.tensor_scalar(p_t[:, :width], p_t[:, :width],
                                        0.0, 0.0, op0=Alu.max, op1=Alu.add,
                                        accum_out=accd_t[:, acol:acol + 1])
            else:
                nc.scalar.activation(p_t[:, :width], p_t[:, :width], Relu,
                                     accum_out=acca_t[:, acol:acol + 1])

        # split outputs: bulk prefix overlaps compute; tiny suffix after the
        # engine's last op
        nc.sync.dma_start(accd_d[:], accd_t[:])
        nc.gpsimd.dma_start(acca_d[:], acca_t[:])

    nc.compile()
    return nc


def _get_program():
    global _COMPILED
    if _COMPILED is None:
        _COMPILED = _build_program()
    return _COMPILED


def _core_blocks(k):
    """17 (row, col) upper-tri blocks for core k; the 2 diagonal first."""
    ra, rb = k, NBLK - 1 - k
    order = [(ra, ra), (rb, rb)]
    order += [(ra, c) for c in range(ra + 1, NBLK)]
    order += [(rb, c) for c in range(rb + 1, NBLK)]
    assert len(order) == NBLOCKS
    return order


def kernel(inputs: np.ndarray, target: np.ndarray) -> np.ndarray:
    global LAST_RESULTS
    x = np.asarray(inputs, dtype=np.float32)
    t = np.asarray(target).astype(np.int64)
    assert x.shape == (N, D) and t.shape == (N,)

    perm = np.argsort(t, kind="stable")
    xs = x[perm]
    ts = t[perm]

    x8 = xs.astype(FP8)                       # [N, 256]
    x8f = x8.astype(np.float32)
    sq = (xs.astype(np.float64) ** 2).sum(axis=1).astype(np.float32)

    h = (-0.5 * sq).astype(np.float32)                      # -sq_j / 2
    c = (0.5 * (MARGIN - sq)).astype(np.float32)            # (m - sq_i)/2
    h_hi = h.astype(FP8)
    h_lo = (h - h_hi.astype(np.float32)).astype(FP8)
    c_hi = c.astype(FP8)
    c_lo = (c - c_hi.astype(np.float32)).astype(FP8)

    # packed operand [128, 2, N]: xop[p, s, n] = x8[n, s*128 + p]
    xop = np.ascontiguousarray(x8.T.reshape(2, P, N).transpose(1, 0, 2))

    in_maps = []
    for k in range(NCORES):
        order = _core_blocks(k)
        ops = np.empty((P, NSLOTS * 2, BLK), FP8)
        aug = np.zeros((2, NBLOCKS * 4, BLK), FP8)
        for pos, (r, cb) in enumerate(order):
            rsl = slice(r * BLK, (r + 1) * BLK)
            csl = slice(cb * BLK, (cb + 1) * BLK)
            ls, rs = _pos_slots(pos)
            ops[:, 2 * ls:2 * ls + 2, :] = xop[:, :, rsl]
            if rs != ls:
                ops[:, 2 * rs:2 * rs + 2, :] = xop[:, :, csl]
            aug[:, 4 * pos, :] = 1.0
            aug[0, 4 * pos + 1, :] = c_hi[rsl]
            aug[1, 4 * pos + 1, :] = c_lo[rsl]
            aug[0, 4 * pos + 2, :] = h_hi[csl]
            aug[1, 4 * pos + 2, :] = h_lo[csl]
            aug[:, 4 * pos + 3, :] = 1.0
        in_maps.append({"ops": ops, "aug": aug})

    nc = _get_program()
    res = bass_utils.run_bass_kernel_spmd(
        nc, in_maps, core_ids=list(range(NCORES)))
    LAST_RESULTS = res

    total = 0.0
    for k in range(NCORES):
        accd = res.results[k]["accd"].astype(np.float64)
        acca = res.results[k]["acca"].astype(np.float64)
        total += (accd.sum(axis=0) * W_DVE).sum()
        total += (acca.sum(axis=0) * W_ACT).sum()

    # exact diagonal-entry removal (device computed i==j in the weight-2
    # diagonal sub-square groups)
    g_ii = (x8f * x8f).sum(axis=1, dtype=np.float32)
    a_ii = (g_ii
            + (h_hi.astype(np.float32) + h_lo.astype(np.float32))
            + (c_hi.astype(np.float32) + c_lo.astype(np.float32)))
    total -= 2.0 * np.maximum(a_ii, 0.0).astype(np.float64).sum()

    # exact same-class term in fp64: sum_{same, i != j} [d - relu(m - d)]
    sq64 = (xs.astype(np.float64) ** 2).sum(axis=1)
    nclasses = int(ts.max()) + 1
    counts = np.bincount(ts, minlength=nclasses)
    starts = np.concatenate([[0], np.cumsum(counts)])
    for cc in range(nclasses):
        lo, hi = starts[cc], starts[cc + 1]
        if hi - lo < 2:
            continue
        Xc = xs[lo:hi].astype(np.float64)
        sqc = sq64[lo:hi]
        dm = sqc[:, None] + sqc[None, :] - 2.0 * (Xc @ Xc.T)
        np.fill_diagonal(dm, np.nan)
        total += np.nansum(dm) - np.nansum(np.maximum(MARGIN - dm, 0.0))

    loss = total / (N * (N - 1.0) * 2.0)
    return np.float32(loss)
